# revision 23
# baseline (speedup 1.0000x reference)
"""GCN layer (support = X @ W; out[r] += val * support[c]; + bias) on 8 trn2 cores.

Sharding: nodes are dest-sharded across the 8 cores (per the sharding hint) —
core c owns dest rows [c*12500, (c+1)*12500), its edges (partitioned by dest
row), and the matching shard of X for the dense matmul.

Launch 1 (SPMD): core c computes its support shard = X_shard @ W in bf16
  (PSUM fp32 accumulate, W stationary, 512-row moving tiles), writing
  support^T back to DRAM.

Host (halo exchange + edge packing): assembles the full support, then per core
  sorts its edges by dest and packs them into 128-edge tiles / GT-tile groups
  (W_G-dest windows) / 4096-slot ops, materializing the per-edge source-row
  stream G = support[col] (the halo-exchange expansion, done host-side: each
  on-device SWDGE gather descriptor costs ~8ns of Q7 time, a ~1.6ms/core floor
  for per-edge gathers, while a sequential stream runs at full DMA bandwidth)
  plus compact per-slot (window-offset, val) metadata.

Launch 2 (SPMD): per op, stream G tiles sequentially (two DMA chunks issued
  from different sequencers so no single sequencer's ~600ns/DMA config time
  serializes); build the one-hot-times-val scatter matrices S on the DVE
  (iota ramp + is_equal + mult against the per-slot metadata — cheaper than
  streaming S from DRAM); PE matmuls G_tile^T @ S_tile accumulate
  out^T[128 feat, W_G-dest windows] in PSUM over each group's GT tiles
  (fusing the val multiply and the segment sum); DVE evacuates PSUM to bf16.

Host: segment-sums straddled window columns per dest (vectorized reduceat),
  adds bias, returns fp32.
"""

import numpy as np
import ml_dtypes

import concourse.bass as bass
import concourse.tile as tile
from concourse import bacc, mybir
from concourse.bass_utils import run_bass_kernel_spmd

# ---------------- problem constants (hardcoded; kernel.py is self-contained)
N_NODES = 100000
IN_F = 256
OUT_F = 128
NCORES = 8
D_PER_CORE = N_NODES // NCORES  # 12500

# launch-1 geometry
ROWS_PAD = 12800  # 25 * 512

# launch-2 geometry (W_G / GT / NOPS are sized from the data in kernel();
# for the reference graph they resolve to W_G=24, GT=2, NOPS=50)
SLOTS_OP = 4096
TILES_OP = SLOTS_OP // 128  # 32
G_CHUNK = 16                # tiles per g-stream DMA chunk

BF16 = mybir.dt.bfloat16
FP32 = mybir.dt.float32
BF = ml_dtypes.bfloat16


def _new_nc():
    return bacc.Bacc("TRN2", target_bir_lowering=False, debug=False)


# ---------------- launch 1: support = X_shard @ W ----------------
def build_support_program():
    nc = _new_nc()
    xt = nc.declare_dram_parameter("xt", [2, 128, ROWS_PAD], BF16, isOutput=False)
    w = nc.declare_dram_parameter("w", [2, 128, OUT_F], BF16, isOutput=False)
    # support written transposed: [128 feat, ROWS_PAD]
    sup = nc.declare_dram_parameter("sup", [OUT_F, ROWS_PAD], BF16, isOutput=True)

    CH = 512  # rows per matmul (rhs free dim; PSUM bank = 512 fp32)
    with tile.TileContext(nc) as tc:
        with (
            tc.tile_pool(name="xt_pool", bufs=1) as xt_pool,
            tc.tile_pool(name="w_pool", bufs=1) as w_pool,
            tc.tile_pool(name="ev_pool", bufs=4) as ev_pool,
            tc.tile_pool(name="ps_pool", bufs=6, space="PSUM") as ps_pool,
        ):
            w_t = w_pool.tile([128, 2, OUT_F], BF16)
            for k in range(2):
                nc.sync.dma_start(w_t[:, k, :], w[k])
            xt_t = xt_pool.tile([128, 2, ROWS_PAD], BF16)
            for i in range(ROWS_PAD // CH):
                for k in range(2):
                    eng = nc.sync if k == 0 else nc.scalar
                    eng.dma_start(
                        xt_t[:, k, CH * i : CH * (i + 1)],
                        xt[k, :, CH * i : CH * (i + 1)],
                    )

            for i in range(ROWS_PAD // CH):
                ps = ps_pool.tile([128, CH], FP32, space="PSUM")
                for k in range(2):
                    nc.tensor.matmul(
                        out=ps[:],
                        lhsT=w_t[:, k, :],
                        rhs=xt_t[:, k, CH * i : CH * (i + 1)],
                        start=(k == 0),
                        stop=(k == 1),
                    )
                ev = ev_pool.tile([128, CH], BF16)
                nc.vector.tensor_copy(ev[:], ps[:])
                nc.gpsimd.dma_start(sup[:, CH * i : CH * (i + 1)], ev[:])
    nc.compile()
    return nc


# ---------------- launch 2: streamed scatter-matmul ----------------
def build_spmm_program(nops, gt, w_g):
    cols_op = (TILES_OP // gt) * w_g
    nc = _new_nc()
    g = nc.declare_dram_parameter("g", [nops, 128, TILES_OP, OUT_F], BF16, isOutput=False)
    # wv[j, lane, t] = dest-window offset of slot (j,t,lane); edge vals are
    # pre-multiplied into g on the host, so S is a pure 0/1 one-hot
    wv = nc.declare_dram_parameter("wv", [nops, 128, TILES_OP], BF16, isOutput=False)
    out = nc.declare_dram_parameter("out", [OUT_F, nops * cols_op], BF16, isOutput=True)

    with tile.TileContext(nc) as tc:
        with (
            tc.tile_pool(name="io_pool", bufs=1) as io_pool,
            tc.tile_pool(name="g_pool", bufs=8) as g_pool,
            tc.tile_pool(name="wv_pool", bufs=6) as wv_pool,
            tc.tile_pool(name="s_pool", bufs=6) as s_pool,
            tc.tile_pool(name="ev_pool", bufs=6) as ev_pool,
            tc.tile_pool(name="ps_pool", bufs=7, space="PSUM") as ps_pool,
        ):
            # one-time [t, w] -> w ramp, bf16
            io_f = io_pool.tile([128, TILES_OP, w_g], FP32)
            nc.gpsimd.iota(io_f[:], [[0, TILES_OP], [1, w_g]], base=0,
                           channel_multiplier=0,
                           allow_small_or_imprecise_dtypes=True)
            io_b = io_pool.tile([128, TILES_OP, w_g], BF16)
            nc.vector.tensor_copy(io_b[:], io_f[:])

            for j in range(nops):
                # g streamed in chunks; DMA issue spread across sequencers
                # (SP + Act) so no single sequencer's ~600ns/DMA config time
                # serializes the stream.
                g_cs = []
                for h in range(TILES_OP // G_CHUNK):
                    g_c = g_pool.tile([128, G_CHUNK, OUT_F], BF16)
                    eng = nc.sync if h % 2 == 0 else nc.scalar
                    eng.dma_start(
                        g_c[:], g[j, :, G_CHUNK * h : G_CHUNK * (h + 1), :]
                    )
                    g_cs.append(g_c)
                wv_t = wv_pool.tile([128, TILES_OP, 1], BF16)
                nc.scalar.dma_start(wv_t[:, :, 0], wv[j])
                # S[lane, t, w] = (w == woff)  -- built on DVE, not DMA'd
                s_t = s_pool.tile([128, TILES_OP, w_g], BF16)
                nc.vector.tensor_tensor(
                    out=s_t[:], in0=io_b[:],
                    in1=wv_t[:].to_broadcast([128, TILES_OP, w_g]),
                    op=mybir.AluOpType.is_equal,
                )

                ps = ps_pool.tile([128, cols_op], FP32, space="PSUM")
                for t in range(TILES_OP):
                    grp = t // gt
                    nc.tensor.matmul(
                        out=ps[:, w_g * grp : w_g * (grp + 1)],
                        lhsT=g_cs[t // G_CHUNK][:, t % G_CHUNK, :],
                        rhs=s_t[:, t, :],
                        start=(t % gt == 0),
                        stop=(t % gt == gt - 1),
                    )
                ev = ev_pool.tile([128, cols_op], BF16)
                nc.scalar.activation(ev[:], ps[:],
                                     mybir.ActivationFunctionType.Identity)
                nc.sync.dma_start(out[:, cols_op * j : cols_op * (j + 1)], ev[:])
    nc.compile()
    return nc


# ---------------- host-side packing ----------------
def _sorted_core_edges(adj_row, adj_col, adj_val):
    """Per-core (dest-local sorted) edge lists."""
    core_of = adj_row // D_PER_CORE
    per_core = []
    for c in range(NCORES):
        m = core_of == c
        d = (adj_row[m] - c * D_PER_CORE).astype(np.int64)
        order = np.argsort(d, kind="stable")
        per_core.append(
            (d[order], adj_col[m].astype(np.int64)[order], adj_val[m][order])
        )
    return per_core


def _choose_geometry(per_core):
    """Size NOPS / GT / W_G from the data (50 / 2 / 24 for the reference
    graph). The program is compiled inside kernel(), so the geometry can
    follow the data."""
    e_max = max(len(d) for d, _, _ in per_core)
    gt, w_g = 2, 32
    for gt_try in (2, 1):
        gsz = gt_try * 128
        span = 0
        for d, _, _ in per_core:
            if not len(d):
                continue
            dp = np.full(-(-len(d) // gsz) * gsz, d[-1], np.int64)
            dp[: len(d)] = d
            seg = dp.reshape(-1, gsz)
            span = max(span, int((seg.max(1) - seg.min(1)).max()) + 1)
        w_try = -(-(span + 1) // 4) * 4
        if w_try <= 32:
            gt, w_g = gt_try, w_try
            break
    assert w_g <= 32, "dest window too wide even at GT=1"
    nops = max(1, -(-e_max // SLOTS_OP))
    return nops, gt, w_g


def _pack_core(d, cl, v, support_bf, nops, gt, w_g):
    """Pack one core's dest-sorted edges into the device arrays."""
    slots = nops * SLOTS_OP
    E = len(d)
    assert E <= slots

    d_pad = np.zeros(slots, np.int64)
    d_pad[:E] = d
    cl_pad = np.zeros(slots, np.int64)
    cl_pad[:E] = cl
    v_pad = np.zeros(slots, np.float32)
    v_pad[:E] = v

    bases = d_pad[:: gt * 128].copy()  # first dest of each group
    w = d_pad - np.repeat(bases, gt * 128)
    assert (w[:E] >= 0).all() and (w[:E] < w_g).all(), (
        f"group window overflow: {w[:E].max()} >= {w_g}"
    )
    w[E:] = 0

    # wv[j, lane, t] = window offset (small ints, exact in bf16)
    wv = np.ascontiguousarray(
        w.reshape(nops, TILES_OP, 128).transpose(0, 2, 1).astype(BF)
    )  # [nops, 128, TILES_OP]

    # pre-scale the halo rows by the edge values (pad slots have val 0)
    g_rows = (
        support_bf[cl_pad].astype(np.float32) * v_pad[:, None]
    ).astype(BF)  # [slots, 128]
    g_arr = np.ascontiguousarray(
        g_rows.reshape(nops, TILES_OP, 128, OUT_F).transpose(0, 2, 1, 3)
    )  # [nops, 128, TILES_OP, OUT_F]
    return g_arr, wv, bases


def kernel(X_input, adj_row, adj_col, adj_val, W, bias):
    X_input = np.asarray(X_input, np.float32)
    adj_row = np.asarray(adj_row)
    adj_col = np.asarray(adj_col)
    adj_val = np.asarray(adj_val, np.float32)
    W = np.asarray(W, np.float32)
    bias = np.asarray(bias, np.float32)

    # ---- launch 1: support shards (bf16)
    w_dev = np.ascontiguousarray(W.astype(BF).reshape(2, 128, OUT_F))
    nc1 = build_support_program()
    in_maps1 = []
    for c in range(NCORES):
        sl = np.zeros((IN_F, ROWS_PAD), np.float32)
        sl[:, :D_PER_CORE] = X_input[c * D_PER_CORE : (c + 1) * D_PER_CORE].T
        xt = np.ascontiguousarray(sl.astype(BF).reshape(2, 128, ROWS_PAD))
        in_maps1.append({"xt": xt, "w": w_dev})
    res1 = run_bass_kernel_spmd(nc1, in_maps1, list(range(NCORES)))
    kernel.last_res1 = res1
    support_bf = np.concatenate(
        [
            np.ascontiguousarray(np.asarray(res1.results[c]["sup"])[:, :D_PER_CORE].T)
            for c in range(NCORES)
        ],
        axis=0,
    )  # [100000, 128] bf16

    # ---- host packing (halo expansion per core)
    per_core = _sorted_core_edges(adj_row, adj_col, adj_val)
    nops, gt, w_g = _choose_geometry(per_core)
    in_maps2 = []
    bases_all = []
    for c in range(NCORES):
        d, cl, v = per_core[c]
        g_arr, wv, bases = _pack_core(d, cl, v, support_bf, nops, gt, w_g)
        in_maps2.append({"g": g_arr, "wv": wv})
        bases_all.append(bases)

    # ---- launch 2
    nc2 = build_spmm_program(nops, gt, w_g)
    res2 = run_bass_kernel_spmd(nc2, in_maps2, list(range(NCORES)))
    kernel.last_res2 = res2

    # ---- unshard: per-dest segment sum over window columns, + bias
    out = np.empty((N_NODES, OUT_F), np.float32)
    n_groups = nops * SLOTS_OP // (gt * 128)
    w_off = np.tile(np.arange(w_g), n_groups)
    for c in range(NCORES):
        oT = np.asarray(res2.results[c]["out"]).astype(np.float32)  # [128, cols]
        cols = oT.T
        dest_of_col = np.clip(np.repeat(bases_all[c], w_g) + w_off, 0, D_PER_CORE - 1)
        ordc = np.argsort(dest_of_col, kind="stable")
        dd = dest_of_col[ordc]
        bnd = np.flatnonzero(np.r_[True, dd[1:] != dd[:-1]])
        sums = np.add.reduceat(cols[ordc], bnd, axis=0)
        acc = np.zeros((D_PER_CORE, OUT_F), np.float32)
        acc[dd[bnd]] = sums
        out[c * D_PER_CORE : (c + 1) * D_PER_CORE] = acc
    return out + bias


# revision 27
# speedup vs baseline: 1.0052x; 1.0052x over previous
"""GCN layer (support = X @ W; out[r] += val * support[c]; + bias) on 8 trn2 cores.

Sharding: nodes are dest-sharded across the 8 cores (per the sharding hint) —
core c owns dest rows [c*12500, (c+1)*12500), its edges (partitioned by dest
row), and the matching shard of X for the dense matmul.

Launch 1 (SPMD): core c computes its support shard = X_shard @ W in bf16
  (PSUM fp32 accumulate, W stationary, 512-row moving tiles), writing
  support^T back to DRAM.

Host (halo exchange + edge packing): assembles the full support, then per core
  sorts its edges by dest and packs them into 128-edge tiles / GT-tile groups
  (W_G-dest windows) / 4096-slot ops, materializing the per-edge source-row
  stream G = support[col] (the halo-exchange expansion, done host-side: each
  on-device SWDGE gather descriptor costs ~8ns of Q7 time, a ~1.6ms/core floor
  for per-edge gathers, while a sequential stream runs at full DMA bandwidth)
  plus compact per-slot (window-offset, val) metadata.

Launch 2 (SPMD): per op, stream G tiles sequentially (two DMA chunks issued
  from different sequencers so no single sequencer's ~600ns/DMA config time
  serializes); build the one-hot-times-val scatter matrices S on the DVE
  (iota ramp + is_equal + mult against the per-slot metadata — cheaper than
  streaming S from DRAM); PE matmuls G_tile^T @ S_tile accumulate
  out^T[128 feat, W_G-dest windows] in PSUM over each group's GT tiles
  (fusing the val multiply and the segment sum); DVE evacuates PSUM to bf16.

Host: segment-sums straddled window columns per dest (vectorized reduceat),
  adds bias, returns fp32.
"""

import numpy as np
import ml_dtypes

import concourse.bass as bass
import concourse.tile as tile
from concourse import bacc, mybir
from concourse.bass_utils import run_bass_kernel_spmd

# ---------------- problem constants (hardcoded; kernel.py is self-contained)
N_NODES = 100000
IN_F = 256
OUT_F = 128
NCORES = 8
D_PER_CORE = N_NODES // NCORES  # 12500

# launch-1 geometry
ROWS_PAD = 12800  # 25 * 512

# launch-2 geometry (W_G / GT / NOPS are sized from the data in kernel();
# for the reference graph they resolve to W_G=24, GT=2, NOPS=50)
SLOTS_OP = 8192
TILES_OP = SLOTS_OP // 128  # 64
G_CHUNK = 16                # tiles per g-stream DMA chunk

BF16 = mybir.dt.bfloat16
FP32 = mybir.dt.float32
BF = ml_dtypes.bfloat16


def _new_nc():
    return bacc.Bacc("TRN2", target_bir_lowering=False, debug=False)


# ---------------- launch 1: support = X_shard @ W ----------------
def build_support_program():
    nc = _new_nc()
    xt = nc.declare_dram_parameter("xt", [2, 128, ROWS_PAD], BF16, isOutput=False)
    w = nc.declare_dram_parameter("w", [2, 128, OUT_F], BF16, isOutput=False)
    # support written transposed: [128 feat, ROWS_PAD]
    sup = nc.declare_dram_parameter("sup", [OUT_F, ROWS_PAD], BF16, isOutput=True)

    CH = 512  # rows per matmul (rhs free dim; PSUM bank = 512 fp32)
    with tile.TileContext(nc) as tc:
        with (
            tc.tile_pool(name="xt_pool", bufs=1) as xt_pool,
            tc.tile_pool(name="w_pool", bufs=1) as w_pool,
            tc.tile_pool(name="ev_pool", bufs=4) as ev_pool,
            tc.tile_pool(name="ps_pool", bufs=4, space="PSUM") as ps_pool,
        ):
            w_t = w_pool.tile([128, 2, OUT_F], BF16)
            for k in range(2):
                nc.sync.dma_start(w_t[:, k, :], w[k])
            xt_t = xt_pool.tile([128, 2, ROWS_PAD], BF16)
            for i in range(ROWS_PAD // CH):
                for k in range(2):
                    eng = nc.sync if k == 0 else nc.scalar
                    eng.dma_start(
                        xt_t[:, k, CH * i : CH * (i + 1)],
                        xt[k, :, CH * i : CH * (i + 1)],
                    )

            for i in range(ROWS_PAD // CH):
                ps = ps_pool.tile([128, CH], FP32, space="PSUM")
                for k in range(2):
                    nc.tensor.matmul(
                        out=ps[:],
                        lhsT=w_t[:, k, :],
                        rhs=xt_t[:, k, CH * i : CH * (i + 1)],
                        start=(k == 0),
                        stop=(k == 1),
                    )
                ev = ev_pool.tile([128, CH], BF16)
                nc.vector.tensor_copy(ev[:], ps[:])
                nc.gpsimd.dma_start(sup[:, CH * i : CH * (i + 1)], ev[:])
    nc.compile()
    return nc


# ---------------- launch 2: streamed scatter-matmul ----------------
def build_spmm_program(nops, gt, w_g):
    cols_op = (TILES_OP // gt) * w_g
    nc = _new_nc()
    g = nc.declare_dram_parameter("g", [nops, 128, TILES_OP, OUT_F], BF16, isOutput=False)
    # wv[j, lane, t] = dest-window offset of slot (j,t,lane); edge vals are
    # pre-multiplied into g on the host, so S is a pure 0/1 one-hot
    wv = nc.declare_dram_parameter("wv", [nops, 128, TILES_OP], BF16, isOutput=False)
    out = nc.declare_dram_parameter("out", [OUT_F, nops * cols_op], BF16, isOutput=True)

    with tile.TileContext(nc) as tc:
        with (
            tc.tile_pool(name="io_pool", bufs=1) as io_pool,
            tc.tile_pool(name="g_pool", bufs=8) as g_pool,
            tc.tile_pool(name="wv_pool", bufs=6) as wv_pool,
            tc.tile_pool(name="s_pool", bufs=6) as s_pool,
            tc.tile_pool(name="ev_pool", bufs=6) as ev_pool,
            tc.tile_pool(name="ps_pool", bufs=4, space="PSUM") as ps_pool,
        ):
            # one-time [t, w] -> w ramp, bf16
            io_f = io_pool.tile([128, TILES_OP, w_g], FP32)
            nc.gpsimd.iota(io_f[:], [[0, TILES_OP], [1, w_g]], base=0,
                           channel_multiplier=0,
                           allow_small_or_imprecise_dtypes=True)
            io_b = io_pool.tile([128, TILES_OP, w_g], BF16)
            nc.vector.tensor_copy(io_b[:], io_f[:])

            for j in range(nops):
                # g streamed in chunks; DMA issue spread across sequencers
                # (SP + Act) so no single sequencer's ~600ns/DMA config time
                # serializes the stream.
                g_cs = []
                for h in range(TILES_OP // G_CHUNK):
                    g_c = g_pool.tile([128, G_CHUNK, OUT_F], BF16)
                    eng = nc.sync if h % 2 == 0 else nc.scalar
                    eng.dma_start(
                        g_c[:], g[j, :, G_CHUNK * h : G_CHUNK * (h + 1), :]
                    )
                    g_cs.append(g_c)
                wv_t = wv_pool.tile([128, TILES_OP, 1], BF16)
                nc.scalar.dma_start(wv_t[:, :, 0], wv[j])
                # S[lane, t, w] = (w == woff)  -- built on DVE, not DMA'd
                s_t = s_pool.tile([128, TILES_OP, w_g], BF16)
                nc.vector.tensor_tensor(
                    out=s_t[:], in0=io_b[:],
                    in1=wv_t[:].to_broadcast([128, TILES_OP, w_g]),
                    op=mybir.AluOpType.is_equal,
                )

                # cols_op > one PSUM bank: split the op across two psum tiles
                half_cols = cols_op // 2
                ps_a = ps_pool.tile([128, half_cols], FP32, space="PSUM")
                ps_b = ps_pool.tile([128, half_cols], FP32, space="PSUM")
                ps_h = [ps_a, ps_b]
                for t in range(TILES_OP):
                    half = t // (TILES_OP // 2)
                    grp = (t % (TILES_OP // 2)) // gt
                    nc.tensor.matmul(
                        out=ps_h[half][:, w_g * grp : w_g * (grp + 1)],
                        lhsT=g_cs[t // G_CHUNK][:, t % G_CHUNK, :],
                        rhs=s_t[:, t, :],
                        start=(t % gt == 0),
                        stop=(t % gt == gt - 1),
                    )
                ev = ev_pool.tile([128, cols_op], BF16)
                for half in range(2):
                    nc.scalar.activation(
                        ev[:, half_cols * half : half_cols * (half + 1)],
                        ps_h[half][:],
                        mybir.ActivationFunctionType.Identity,
                    )
                nc.sync.dma_start(out[:, cols_op * j : cols_op * (j + 1)], ev[:])
    nc.compile()
    return nc


# ---------------- host-side packing ----------------
def _sorted_core_edges(adj_row, adj_col, adj_val):
    """Per-core (dest-local sorted) edge lists."""
    core_of = adj_row // D_PER_CORE
    per_core = []
    for c in range(NCORES):
        m = core_of == c
        d = (adj_row[m] - c * D_PER_CORE).astype(np.int64)
        order = np.argsort(d, kind="stable")
        per_core.append(
            (d[order], adj_col[m].astype(np.int64)[order], adj_val[m][order])
        )
    return per_core


def _choose_geometry(per_core):
    """Size NOPS / GT / W_G from the data (50 / 2 / 24 for the reference
    graph). The program is compiled inside kernel(), so the geometry can
    follow the data."""
    e_max = max(len(d) for d, _, _ in per_core)
    gt, w_g = 2, 32
    for gt_try in (2, 1):
        gsz = gt_try * 128
        span = 0
        for d, _, _ in per_core:
            if not len(d):
                continue
            dp = np.full(-(-len(d) // gsz) * gsz, d[-1], np.int64)
            dp[: len(d)] = d
            seg = dp.reshape(-1, gsz)
            span = max(span, int((seg.max(1) - seg.min(1)).max()) + 1)
        w_try = -(-(span + 1) // 4) * 4
        if w_try <= 32:
            gt, w_g = gt_try, w_try
            break
    assert w_g <= 32, "dest window too wide even at GT=1"
    nops = max(1, -(-e_max // SLOTS_OP))
    return nops, gt, w_g


def _pack_core(d, cl, v, support_bf, nops, gt, w_g):
    """Pack one core's dest-sorted edges into the device arrays."""
    slots = nops * SLOTS_OP
    E = len(d)
    assert E <= slots

    d_pad = np.zeros(slots, np.int64)
    d_pad[:E] = d
    cl_pad = np.zeros(slots, np.int64)
    cl_pad[:E] = cl
    v_pad = np.zeros(slots, np.float32)
    v_pad[:E] = v

    bases = d_pad[:: gt * 128].copy()  # first dest of each group
    w = d_pad - np.repeat(bases, gt * 128)
    assert (w[:E] >= 0).all() and (w[:E] < w_g).all(), (
        f"group window overflow: {w[:E].max()} >= {w_g}"
    )
    w[E:] = 0

    # wv[j, lane, t] = window offset (small ints, exact in bf16)
    wv = np.ascontiguousarray(
        w.reshape(nops, TILES_OP, 128).transpose(0, 2, 1).astype(BF)
    )  # [nops, 128, TILES_OP]

    # pre-scale the halo rows by the edge values (pad slots have val 0)
    g_rows = (
        support_bf[cl_pad].astype(np.float32) * v_pad[:, None]
    ).astype(BF)  # [slots, 128]
    g_arr = np.ascontiguousarray(
        g_rows.reshape(nops, TILES_OP, 128, OUT_F).transpose(0, 2, 1, 3)
    )  # [nops, 128, TILES_OP, OUT_F]
    return g_arr, wv, bases


def kernel(X_input, adj_row, adj_col, adj_val, W, bias):
    X_input = np.asarray(X_input, np.float32)
    adj_row = np.asarray(adj_row)
    adj_col = np.asarray(adj_col)
    adj_val = np.asarray(adj_val, np.float32)
    W = np.asarray(W, np.float32)
    bias = np.asarray(bias, np.float32)

    # ---- launch 1: support shards (bf16)
    w_dev = np.ascontiguousarray(W.astype(BF).reshape(2, 128, OUT_F))
    nc1 = build_support_program()
    in_maps1 = []
    for c in range(NCORES):
        sl = np.zeros((IN_F, ROWS_PAD), np.float32)
        sl[:, :D_PER_CORE] = X_input[c * D_PER_CORE : (c + 1) * D_PER_CORE].T
        xt = np.ascontiguousarray(sl.astype(BF).reshape(2, 128, ROWS_PAD))
        in_maps1.append({"xt": xt, "w": w_dev})
    res1 = run_bass_kernel_spmd(nc1, in_maps1, list(range(NCORES)))
    kernel.last_res1 = res1
    support_bf = np.concatenate(
        [
            np.ascontiguousarray(np.asarray(res1.results[c]["sup"])[:, :D_PER_CORE].T)
            for c in range(NCORES)
        ],
        axis=0,
    )  # [100000, 128] bf16

    # ---- host packing (halo expansion per core)
    per_core = _sorted_core_edges(adj_row, adj_col, adj_val)
    nops, gt, w_g = _choose_geometry(per_core)
    in_maps2 = []
    bases_all = []
    for c in range(NCORES):
        d, cl, v = per_core[c]
        g_arr, wv, bases = _pack_core(d, cl, v, support_bf, nops, gt, w_g)
        in_maps2.append({"g": g_arr, "wv": wv})
        bases_all.append(bases)

    # ---- launch 2
    nc2 = build_spmm_program(nops, gt, w_g)
    res2 = run_bass_kernel_spmd(nc2, in_maps2, list(range(NCORES)))
    kernel.last_res2 = res2

    # ---- unshard: per-dest segment sum over window columns, + bias
    out = np.empty((N_NODES, OUT_F), np.float32)
    n_groups = nops * SLOTS_OP // (gt * 128)
    w_off = np.tile(np.arange(w_g), n_groups)
    for c in range(NCORES):
        oT = np.asarray(res2.results[c]["out"]).astype(np.float32)  # [128, cols]
        cols = oT.T
        dest_of_col = np.clip(np.repeat(bases_all[c], w_g) + w_off, 0, D_PER_CORE - 1)
        ordc = np.argsort(dest_of_col, kind="stable")
        dd = dest_of_col[ordc]
        bnd = np.flatnonzero(np.r_[True, dd[1:] != dd[:-1]])
        sums = np.add.reduceat(cols[ordc], bnd, axis=0)
        acc = np.zeros((D_PER_CORE, OUT_F), np.float32)
        acc[dd[bnd]] = sums
        out[c * D_PER_CORE : (c + 1) * D_PER_CORE] = acc
    return out + bias


# revision 28
# speedup vs baseline: 1.0061x; 1.0009x over previous
"""GCN layer (support = X @ W; out[r] += val * support[c]; + bias) on 8 trn2 cores.

Sharding: nodes are dest-sharded across the 8 cores (per the sharding hint) —
core c owns dest rows [c*12500, (c+1)*12500), its edges (partitioned by dest
row), and the matching shard of X for the dense matmul.

Launch 1 (SPMD): core c computes its support shard = X_shard @ W in bf16
  (PSUM fp32 accumulate, W stationary, 512-row moving tiles), writing
  support^T back to DRAM.

Host (halo exchange + edge packing): assembles the full support, then per core
  sorts its edges by dest and packs them into 128-edge tiles / GT-tile groups
  (W_G-dest windows) / 4096-slot ops, materializing the per-edge source-row
  stream G = support[col] (the halo-exchange expansion, done host-side: each
  on-device SWDGE gather descriptor costs ~8ns of Q7 time, a ~1.6ms/core floor
  for per-edge gathers, while a sequential stream runs at full DMA bandwidth)
  plus compact per-slot (window-offset, val) metadata.

Launch 2 (SPMD): per op, stream G tiles sequentially (two DMA chunks issued
  from different sequencers so no single sequencer's ~600ns/DMA config time
  serializes); build the one-hot-times-val scatter matrices S on the DVE
  (iota ramp + is_equal + mult against the per-slot metadata — cheaper than
  streaming S from DRAM); PE matmuls G_tile^T @ S_tile accumulate
  out^T[128 feat, W_G-dest windows] in PSUM over each group's GT tiles
  (fusing the val multiply and the segment sum); DVE evacuates PSUM to bf16.

Host: segment-sums straddled window columns per dest (vectorized reduceat),
  adds bias, returns fp32.
"""

import numpy as np
import ml_dtypes

import concourse.bass as bass
import concourse.tile as tile
from concourse import bacc, mybir
from concourse.bass_utils import run_bass_kernel_spmd

# ---------------- problem constants (hardcoded; kernel.py is self-contained)
N_NODES = 100000
IN_F = 256
OUT_F = 128
NCORES = 8
D_PER_CORE = N_NODES // NCORES  # 12500

# launch-1 geometry
ROWS_PAD = 12800  # 25 * 512

# launch-2 geometry (W_G / GT / NOPS are sized from the data in kernel();
# for the reference graph they resolve to W_G=24, GT=2, NOPS=50)
SLOTS_OP = 8192
TILES_OP = SLOTS_OP // 128  # 64
G_CHUNK = 16                # tiles per g-stream DMA chunk

BF16 = mybir.dt.bfloat16
FP32 = mybir.dt.float32
BF = ml_dtypes.bfloat16


def _new_nc():
    return bacc.Bacc("TRN2", target_bir_lowering=False, debug=False)


# ---------------- launch 1: support = X_shard @ W ----------------
def build_support_program():
    nc = _new_nc()
    xt = nc.declare_dram_parameter("xt", [2, 128, ROWS_PAD], BF16, isOutput=False)
    w = nc.declare_dram_parameter("w", [2, 128, OUT_F], BF16, isOutput=False)
    # support written transposed: [128 feat, ROWS_PAD]
    sup = nc.declare_dram_parameter("sup", [OUT_F, ROWS_PAD], BF16, isOutput=True)

    CH = 512  # rows per matmul (rhs free dim; PSUM bank = 512 fp32)
    with tile.TileContext(nc) as tc:
        with (
            tc.tile_pool(name="xt_pool", bufs=1) as xt_pool,
            tc.tile_pool(name="w_pool", bufs=1) as w_pool,
            tc.tile_pool(name="ev_pool", bufs=4) as ev_pool,
            tc.tile_pool(name="ps_pool", bufs=4, space="PSUM") as ps_pool,
        ):
            w_t = w_pool.tile([128, 2, OUT_F], BF16)
            for k in range(2):
                nc.sync.dma_start(w_t[:, k, :], w[k])
            xt_t = xt_pool.tile([128, 2, ROWS_PAD], BF16)
            for i in range(ROWS_PAD // CH):
                for k in range(2):
                    eng = nc.sync if k == 0 else nc.scalar
                    eng.dma_start(
                        xt_t[:, k, CH * i : CH * (i + 1)],
                        xt[k, :, CH * i : CH * (i + 1)],
                    )

            # batch 4 chunk evacs into one ev tile -> one sup write each,
            # so the Q7's ~1us fixed cost per SWDGE write stops gating the loop
            EB = 4
            n_ch = ROWS_PAD // CH
            for i in range(n_ch):
                ps = ps_pool.tile([128, CH], FP32, space="PSUM")
                for k in range(2):
                    nc.tensor.matmul(
                        out=ps[:],
                        lhsT=w_t[:, k, :],
                        rhs=xt_t[:, k, CH * i : CH * (i + 1)],
                        start=(k == 0),
                        stop=(k == 1),
                    )
                if i % EB == 0:
                    ev = ev_pool.tile([128, EB, CH], BF16)
                    ev0 = i
                nc.vector.tensor_copy(ev[:, i - ev0, :], ps[:])
                if i - ev0 == EB - 1 or i == n_ch - 1:
                    nb = i - ev0 + 1
                    nc.gpsimd.dma_start(
                        sup[:, CH * ev0 : CH * (ev0 + nb)], ev[:, :nb, :]
                    )
    nc.compile()
    return nc


# ---------------- launch 2: streamed scatter-matmul ----------------
def build_spmm_program(nops, gt, w_g):
    cols_op = (TILES_OP // gt) * w_g
    nc = _new_nc()
    g = nc.declare_dram_parameter("g", [nops, 128, TILES_OP, OUT_F], BF16, isOutput=False)
    # wv[j, lane, t] = dest-window offset of slot (j,t,lane); edge vals are
    # pre-multiplied into g on the host, so S is a pure 0/1 one-hot
    wv = nc.declare_dram_parameter("wv", [nops, 128, TILES_OP], BF16, isOutput=False)
    out = nc.declare_dram_parameter("out", [OUT_F, nops * cols_op], BF16, isOutput=True)

    with tile.TileContext(nc) as tc:
        with (
            tc.tile_pool(name="io_pool", bufs=1) as io_pool,
            tc.tile_pool(name="g_pool", bufs=8) as g_pool,
            tc.tile_pool(name="wv_pool", bufs=6) as wv_pool,
            tc.tile_pool(name="s_pool", bufs=6) as s_pool,
            tc.tile_pool(name="ev_pool", bufs=6) as ev_pool,
            tc.tile_pool(name="ps_pool", bufs=4, space="PSUM") as ps_pool,
        ):
            # one-time [t, w] -> w ramp, bf16
            io_f = io_pool.tile([128, TILES_OP, w_g], FP32)
            nc.gpsimd.iota(io_f[:], [[0, TILES_OP], [1, w_g]], base=0,
                           channel_multiplier=0,
                           allow_small_or_imprecise_dtypes=True)
            io_b = io_pool.tile([128, TILES_OP, w_g], BF16)
            nc.vector.tensor_copy(io_b[:], io_f[:])

            for j in range(nops):
                # g streamed in chunks; DMA issue spread across sequencers
                # (SP + Act) so no single sequencer's ~600ns/DMA config time
                # serializes the stream.
                g_cs = []
                for h in range(TILES_OP // G_CHUNK):
                    g_c = g_pool.tile([128, G_CHUNK, OUT_F], BF16)
                    eng = nc.sync if h % 2 == 0 else nc.scalar
                    eng.dma_start(
                        g_c[:], g[j, :, G_CHUNK * h : G_CHUNK * (h + 1), :]
                    )
                    g_cs.append(g_c)
                wv_t = wv_pool.tile([128, TILES_OP, 1], BF16)
                nc.scalar.dma_start(wv_t[:, :, 0], wv[j])
                # S[lane, t, w] = (w == woff)  -- built on DVE, not DMA'd
                s_t = s_pool.tile([128, TILES_OP, w_g], BF16)
                nc.vector.tensor_tensor(
                    out=s_t[:], in0=io_b[:],
                    in1=wv_t[:].to_broadcast([128, TILES_OP, w_g]),
                    op=mybir.AluOpType.is_equal,
                )

                # cols_op > one PSUM bank: split the op across two psum tiles
                half_cols = cols_op // 2
                ps_a = ps_pool.tile([128, half_cols], FP32, space="PSUM")
                ps_b = ps_pool.tile([128, half_cols], FP32, space="PSUM")
                ps_h = [ps_a, ps_b]
                for t in range(TILES_OP):
                    half = t // (TILES_OP // 2)
                    grp = (t % (TILES_OP // 2)) // gt
                    nc.tensor.matmul(
                        out=ps_h[half][:, w_g * grp : w_g * (grp + 1)],
                        lhsT=g_cs[t // G_CHUNK][:, t % G_CHUNK, :],
                        rhs=s_t[:, t, :],
                        start=(t % gt == 0),
                        stop=(t % gt == gt - 1),
                    )
                ev = ev_pool.tile([128, cols_op], BF16)
                for half in range(2):
                    nc.scalar.activation(
                        ev[:, half_cols * half : half_cols * (half + 1)],
                        ps_h[half][:],
                        mybir.ActivationFunctionType.Identity,
                    )
                nc.sync.dma_start(out[:, cols_op * j : cols_op * (j + 1)], ev[:])
    nc.compile()
    return nc


# ---------------- host-side packing ----------------
def _sorted_core_edges(adj_row, adj_col, adj_val):
    """Per-core (dest-local sorted) edge lists."""
    core_of = adj_row // D_PER_CORE
    per_core = []
    for c in range(NCORES):
        m = core_of == c
        d = (adj_row[m] - c * D_PER_CORE).astype(np.int64)
        order = np.argsort(d, kind="stable")
        per_core.append(
            (d[order], adj_col[m].astype(np.int64)[order], adj_val[m][order])
        )
    return per_core


def _choose_geometry(per_core):
    """Size NOPS / GT / W_G from the data (50 / 2 / 24 for the reference
    graph). The program is compiled inside kernel(), so the geometry can
    follow the data."""
    e_max = max(len(d) for d, _, _ in per_core)
    gt, w_g = 2, 32
    for gt_try in (2, 1):
        gsz = gt_try * 128
        span = 0
        for d, _, _ in per_core:
            if not len(d):
                continue
            dp = np.full(-(-len(d) // gsz) * gsz, d[-1], np.int64)
            dp[: len(d)] = d
            seg = dp.reshape(-1, gsz)
            span = max(span, int((seg.max(1) - seg.min(1)).max()) + 1)
        w_try = -(-(span + 1) // 4) * 4
        if w_try <= 32:
            gt, w_g = gt_try, w_try
            break
    assert w_g <= 32, "dest window too wide even at GT=1"
    nops = max(1, -(-e_max // SLOTS_OP))
    return nops, gt, w_g


def _pack_core(d, cl, v, support_bf, nops, gt, w_g):
    """Pack one core's dest-sorted edges into the device arrays."""
    slots = nops * SLOTS_OP
    E = len(d)
    assert E <= slots

    d_pad = np.zeros(slots, np.int64)
    d_pad[:E] = d
    cl_pad = np.zeros(slots, np.int64)
    cl_pad[:E] = cl
    v_pad = np.zeros(slots, np.float32)
    v_pad[:E] = v

    bases = d_pad[:: gt * 128].copy()  # first dest of each group
    w = d_pad - np.repeat(bases, gt * 128)
    assert (w[:E] >= 0).all() and (w[:E] < w_g).all(), (
        f"group window overflow: {w[:E].max()} >= {w_g}"
    )
    w[E:] = 0

    # wv[j, lane, t] = window offset (small ints, exact in bf16)
    wv = np.ascontiguousarray(
        w.reshape(nops, TILES_OP, 128).transpose(0, 2, 1).astype(BF)
    )  # [nops, 128, TILES_OP]

    # pre-scale the halo rows by the edge values (pad slots have val 0)
    g_rows = (
        support_bf[cl_pad].astype(np.float32) * v_pad[:, None]
    ).astype(BF)  # [slots, 128]
    g_arr = np.ascontiguousarray(
        g_rows.reshape(nops, TILES_OP, 128, OUT_F).transpose(0, 2, 1, 3)
    )  # [nops, 128, TILES_OP, OUT_F]
    return g_arr, wv, bases


def kernel(X_input, adj_row, adj_col, adj_val, W, bias):
    X_input = np.asarray(X_input, np.float32)
    adj_row = np.asarray(adj_row)
    adj_col = np.asarray(adj_col)
    adj_val = np.asarray(adj_val, np.float32)
    W = np.asarray(W, np.float32)
    bias = np.asarray(bias, np.float32)

    # ---- launch 1: support shards (bf16)
    w_dev = np.ascontiguousarray(W.astype(BF).reshape(2, 128, OUT_F))
    nc1 = build_support_program()
    in_maps1 = []
    for c in range(NCORES):
        sl = np.zeros((IN_F, ROWS_PAD), np.float32)
        sl[:, :D_PER_CORE] = X_input[c * D_PER_CORE : (c + 1) * D_PER_CORE].T
        xt = np.ascontiguousarray(sl.astype(BF).reshape(2, 128, ROWS_PAD))
        in_maps1.append({"xt": xt, "w": w_dev})
    res1 = run_bass_kernel_spmd(nc1, in_maps1, list(range(NCORES)))
    kernel.last_res1 = res1
    support_bf = np.concatenate(
        [
            np.ascontiguousarray(np.asarray(res1.results[c]["sup"])[:, :D_PER_CORE].T)
            for c in range(NCORES)
        ],
        axis=0,
    )  # [100000, 128] bf16

    # ---- host packing (halo expansion per core)
    per_core = _sorted_core_edges(adj_row, adj_col, adj_val)
    nops, gt, w_g = _choose_geometry(per_core)
    in_maps2 = []
    bases_all = []
    for c in range(NCORES):
        d, cl, v = per_core[c]
        g_arr, wv, bases = _pack_core(d, cl, v, support_bf, nops, gt, w_g)
        in_maps2.append({"g": g_arr, "wv": wv})
        bases_all.append(bases)

    # ---- launch 2
    nc2 = build_spmm_program(nops, gt, w_g)
    res2 = run_bass_kernel_spmd(nc2, in_maps2, list(range(NCORES)))
    kernel.last_res2 = res2

    # ---- unshard: per-dest segment sum over window columns, + bias
    out = np.empty((N_NODES, OUT_F), np.float32)
    n_groups = nops * SLOTS_OP // (gt * 128)
    w_off = np.tile(np.arange(w_g), n_groups)
    for c in range(NCORES):
        oT = np.asarray(res2.results[c]["out"]).astype(np.float32)  # [128, cols]
        cols = oT.T
        dest_of_col = np.clip(np.repeat(bases_all[c], w_g) + w_off, 0, D_PER_CORE - 1)
        ordc = np.argsort(dest_of_col, kind="stable")
        dd = dest_of_col[ordc]
        bnd = np.flatnonzero(np.r_[True, dd[1:] != dd[:-1]])
        sums = np.add.reduceat(cols[ordc], bnd, axis=0)
        acc = np.zeros((D_PER_CORE, OUT_F), np.float32)
        acc[dd[bnd]] = sums
        out[c * D_PER_CORE : (c + 1) * D_PER_CORE] = acc
    return out + bias
